# revision 55
# baseline (speedup 1.0000x reference)
"""BLOOM attention layer on 8 Trainium2 NeuronCores.

Sharding: tensor-parallel over heads (4 heads/core) x data-parallel over batch
(B=2), mesh [DP=2, TP=4].  Core c handles batch b=c//4, heads 4*(c%4)..4*(c%4)+3.

Per-core device kernel.  All big GEMMs run fp8(e4m3) DoubleRow (2 contraction
rows/cycle): QKV projection, V projection, the full-block ctx matmuls, the
softmax-sum matmuls, and the dense projection.  Scores stay bf16 (contraction
is a single 128 tile -- DoubleRow can't pack it -- and exp() amplifies fp8
error).  fp8 scaling: W_qkv/W_dense are pre-scaled x32 on host so sigma~0.7
lands mid-range e4m3 (raw sigma 0.022 would drown in subnormals); the x32*x32
is unwound in the Exp scale (INV_NORM/1024) and the dense drain (x 1/1024).

Early-query accuracy: rows q<128 average too few softmax terms for fp8 error
to cancel (measured 0.22 abs err all-fp8 vs 0.04 for q>=512), so a thin bf16
shadow path covers exactly those rows: V s-tile 0 is re-projected in bf16
(vb), E/ctx[q<128] stay bf16 (eb/ctxb), and the so=0 dense tile runs bf16
against an unscaled bf16 Wd copy.  Everything else keeps fp8 throughput.

  1. QKV projection from pre-transposed fp8 activations Xt (host pre-arranges
       all fp8 operands into SBUF layout so DMA bursts are 1-8KB contiguous;
       natural [H,S] fp8 gave 512B bursts at ~60GB/s and stalled phase 1).
       Weights packed [128, hp, 2, cols]: each DoubleRow matmul contracts 256
       rows.  Qt,Kt transposed [d, S] (x32 scale), V natural [S, d] fp8.
       qk biases (x32) via ACT-Identity bias; V bias folded into the host-side
       output bias (softmax passes it through).  Slice 0 runs hp-outer over 8
       PSUM banks so compute starts early in the wqk/xt DMA stream.
  2. Per head, per 512-wide query slice (qs descending): per key tile kt,
       st[k,q] = Kt_tile^T matmul over the causal q-suffix only (bf16),
       E = exp(st*(INV_NORM/1024) + alibi_k) -> fp8,
       shared 128x128 lower-triangle mask multiply on diagonal blocks,
       full-block pairs: ctx^T[d,q] += V_pair^T @ E_pair fp8 DoubleRow,
       sums[q] += ones @ E_pair fp8 DoubleRow, with a 2-pair emission lag so
       the ACT exp pipeline stays ahead of the tensor queue,
       ctx^T *= reciprocal_approx_fast(sums) -> fp8 [x32 scale].
  3. Dense partial fp8 DoubleRow over ct pairs: out[s,h'] = ctx8^T^T @ Wd8,
       drain scaled by 1/1024 -> bf16 DRAM.  Runs inside the phase-2 PSUM
       pool scope (no pool-transition barrier), descending so-order so the
       first tiles depend on long-finished qs=3 ctx, output tiles rotating
       three DMA queues (sync/ACT HWDGE + gpsimd SWDGE).
Host: shard/pre-transpose/cast inputs, sum the 4 TP partials per batch and
add (b_dense + b_v @ W_dense) + residual.

Measured on 8 trn2 cores: 227.0us, rel err 1.585e-2 (gate 2e-2; inputs are
fixed-seed so the result is deterministic).  Baseline bf16 kernel: 330.7us.
QK_REQUAL=True adds a bf16 requalify of qkt cols 0:128 (+9.5us, err 8.7e-3)
if more margin is ever needed.
"""

import numpy as np
import ml_dtypes

bf16 = ml_dtypes.bfloat16
f8e4 = ml_dtypes.float8_e4m3

B, S, H, NH = 2, 2048, 2048, 16
HD = H // NH  # 128
INV_NORM = 1.0 / float(np.sqrt(HD))
NCORES = 8
TP = 4
HPC = NH // TP  # heads per core = 4
QSL = 512      # query slice width
KTL = 128      # key tile length
N_QS = S // QSL   # 4
N_KT = S // KTL   # 16
N_HT = H // 128   # 16 contraction tiles for QKV proj
HP = N_HT // 2    # 8 DoubleRow contraction pairs
CPD = HPC // 2    # 2 dense contraction pairs
NCI = 2 * HPC     # 8 qk column tiles

WS = 32.0                   # fp8 weight pre-scale
ESC = INV_NORM / (WS * WS)  # exp scale compensating q,k x32
DSC = 1.0 / (WS * WS)       # dense drain scale compensating ctx,Wd x32
QK_REQUAL = False           # bf16 requalify of qkt cols 0:128 (~9us, ~2x margin)

_program_cache: dict = {}


def _build_program():
    import concourse.tile as tile
    import concourse.mybir as mybir
    from concourse import bacc

    f32 = mybir.dt.float32
    bf = mybir.dt.bfloat16
    f8 = mybir.dt.float8e4
    AFT = mybir.ActivationFunctionType
    DR = mybir.MatmulPerfMode.DoubleRow

    nc = bacc.Bacc(
        "TRN2",
        target_bir_lowering=False,
        debug=False,
        enable_asserts=False,
        num_devices=NCORES,
    )
    # fp8 operands arrive pre-arranged in SBUF layout (host transposes) so
    # every DMA reads >=1KB contiguous DRAM bursts; the natural [H, S] layout
    # would give 512B fp8 bursts (~60 GB/s measured, stalls phase 1)
    xt_d = nc.dram_tensor("xt", [128, N_QS * HP * 2 * QSL], f8, kind="ExternalInput")
    wqk_d = nc.dram_tensor("wqk", [128, HP * 2 * NCI * 128], f8, kind="ExternalInput")
    wv_d = nc.dram_tensor("wv", [128, HP * 2 * HPC * 128], f8, kind="ExternalInput")
    wd_d = nc.dram_tensor("wd", [HPC * 128, H], f8, kind="ExternalInput")
    xtb_d = nc.dram_tensor("xtb", [H, 128], bf, kind="ExternalInput")
    wqkb_d = nc.dram_tensor("wqkb", [H, NCI * 128], bf, kind="ExternalInput")
    wvb_d = nc.dram_tensor("wvb", [H, HPC * 128], bf, kind="ExternalInput")
    wdb_d = nc.dram_tensor("wdb", [HPC * 128, H], bf, kind="ExternalInput")
    bqk_d = nc.dram_tensor("bqk", [128, NCI], f32, kind="ExternalInput")
    alibi_d = nc.dram_tensor("alibi", [128, HPC * N_KT], f32, kind="ExternalInput")
    tri_d = nc.dram_tensor("tri", [128, 128], bf, kind="ExternalInput")
    out_d = nc.dram_tensor("out", [S, H], bf, kind="ExternalOutput")

    xt_r = xt_d.rearrange("p (ss hp two s) -> p ss hp two s", ss=N_QS, hp=HP, two=2)
    wqk_r = wqk_d.rearrange("p (hp two c) -> p hp two c", hp=HP, two=2)
    wv_r = wv_d.rearrange("p (hp two c) -> p hp two c", hp=HP, two=2)
    wd_r = wd_d.rearrange("(cp two p) h -> p cp two h", p=128, two=2)
    xtb_r = xtb_d.rearrange("(ho p) s -> p ho s", p=128)      # [128,16,128]
    wqkb_r = wqkb_d.rearrange("(ho p) c -> p ho c", p=128)    # [128,16,1024]
    wvb_r = wvb_d.rearrange("(ho p) c -> p ho c", p=128)      # [128,16,512]
    wdb_r = wdb_d.rearrange("(co p) h -> p co h", p=128)      # [128,4,2048]
    out_r = out_d.rearrange("(so p) h -> p so h", p=128)      # [128,16,2048]

    with tile.TileContext(nc) as tc:
        with (
            tc.tile_pool(name="singles", bufs=1) as singles,
            tc.tile_pool(name="epool", bufs=8) as epool,        # E tiles
            tc.tile_pool(name="rpool", bufs=2) as rpool,        # recipb
            tc.tile_pool(name="outstage", bufs=3) as outstage,
        ):
            # long-lived SBUF residents (survive all phases)
            qkt_sb = singles.tile([128, NCI, S], bf, tag="qkt_sb", name="qkt_sb")
            v_sb = singles.tile([128, N_KT, HPC * 128], f8, tag="v_sb", name="v_sb")
            ctx_sb = singles.tile([128, HPC, S], f8, tag="ctx_sb", name="ctx_sb")
            # bf16 shadow path for query rows 0-127 (their softmax support is
            # tiny, so fp8's 3% relative error doesn't average out there):
            # v(kt=0), E/ctx(q<128), and the so=0 dense run in bf16
            vb_sb = singles.tile([128, HPC * 128], bf, tag="vb_sb", name="vb_sb")
            ctxb_sb = singles.tile([128, HPC, 128], bf, tag="ctxb_sb", name="ctxb_sb")
            ones_bf = singles.tile([128, 128], bf, tag="ones_bf", name="ones_bf")
            bqk_sb = singles.tile([128, NCI], f32, tag="bqk_sb", name="bqk_sb")
            alibi_sb = singles.tile([128, HPC * N_KT], f32, tag="alibi_sb", name="alibi_sb")
            tri_sb = singles.tile([128, 128], bf, tag="tri_sb", name="tri_sb")
            ones_f8 = singles.tile([128, 2, 128], f8, tag="ones_f8", name="ones_f8")

            with tc.tile_pool(name="p1pool", bufs=1) as p1pool:
                xt_sb = p1pool.tile([128, HP, 2, S], f8, tag="xt_sb", name="xt_sb")
                wqk_sb = p1pool.tile([128, HP, 2, NCI * 128], f8, tag="wqk_sb", name="wqk_sb")
                wv_sb = p1pool.tile([128, HP, 2, HPC * 128], f8, tag="wv_sb", name="wv_sb")
                xtb_sb = p1pool.tile([128, N_HT, 128], bf, tag="xtb_sb", name="xtb_sb")
                wvb_sb = p1pool.tile([128, N_HT, HPC * 128], bf, tag="wvb_sb", name="wvb_sb")
                wqkb_sb = p1pool.tile([128, N_HT, NCI * 128], bf, tag="wqkb_sb", name="wqkb_sb")

                # startup: interleave wqk + xt-slice0 DMAs on both HWDGE
                # queues, hp-row granularity so the hp-outer slice-0 compute
                # can start after the first ~0.4 MB
                nc.scalar.dma_start(out=wqk_sb[:, 0], in_=wqk_r[:, 0])
                nc.sync.dma_start(out=xt_sb[:, 0, :, 0:QSL], in_=xt_r[:, 0, 0])
                for hp in range(1, 4):
                    nc.scalar.dma_start(out=wqk_sb[:, hp], in_=wqk_r[:, hp])
                    nc.sync.dma_start(
                        out=xt_sb[:, hp, :, 0:QSL], in_=xt_r[:, 0, hp]
                    )
                nc.scalar.dma_start(out=bqk_sb, in_=bqk_d[:])
                for hp in range(4, HP):
                    nc.scalar.dma_start(out=wqk_sb[:, hp], in_=wqk_r[:, hp])
                    nc.sync.dma_start(
                        out=xt_sb[:, hp, :, 0:QSL], in_=xt_r[:, 0, hp]
                    )
                nc.vector.memset(ones_f8, 1.0)
                nc.vector.memset(ones_bf, 1.0)

                # ---- phase 1a: Qt/Kt slice 0 over 8 PSUM banks: hp-outer for
                # the first HSPLIT pairs (starts after one hp-row of DMA),
                # then per-ci chains so the 8 bank-stops stagger and their
                # drains (alternating ACT/DVE) overlap the remaining matmuls
                HSPLIT = 5
                scratch = p1pool.tile([128, QSL], bf, tag="scratch", name="scratch")
                nc.vector.memset(scratch, 0.0)
                with tc.tile_pool(name="ps_p1a", bufs=8, space="PSUM") as ps_p1a:
                    # HAM warmup: dummy matmuls fill the DMA-wait idle so the
                    # PE clock gate opens before real work arrives
                    ps_w = ps_p1a.tile([128, QSL], f32, tag="psp1a", name="warmup")
                    for i in range(16):
                        nc.tensor.matmul(
                            ps_w,
                            lhsT=scratch[:, 0:128],
                            rhs=scratch,
                            start=(i == 0),
                            stop=(i == 15),
                        )
                    ps0 = [
                        ps_p1a.tile([128, QSL], f32, tag="psp1a", name=f"qk0_{ci}")
                        for ci in range(NCI)
                    ]
                    for hp in range(HSPLIT):
                        for ci in range(NCI):
                            nc.tensor.matmul(
                                ps0[ci],
                                lhsT=wqk_sb[:, hp, :, ci * 128:(ci + 1) * 128],
                                rhs=xt_sb[:, hp, :, 0:QSL],
                                start=(hp == 0),
                                stop=False,
                                perf_mode=DR,
                            )
                    # remaining bulk DMAs stream behind the slice-0 compute
                    for ss in range(1, N_QS):
                        nc.sync.dma_start(
                            out=xt_sb[:, :, :, ss * QSL:(ss + 1) * QSL],
                            in_=xt_r[:, ss],
                        )
                    nc.sync.dma_start(out=alibi_sb, in_=alibi_d[:])
                    nc.sync.dma_start(out=tri_sb, in_=tri_d[:])
                    nc.sync.dma_start(out=wv_sb, in_=wv_r)
                    nc.sync.dma_start(out=wvb_sb, in_=wvb_r)
                    # bf16 early-path operands ride the ACT HWDGE queue,
                    # which is idle after the wqk startup chunks
                    nc.scalar.dma_start(out=xtb_sb, in_=xtb_r)
                    if QK_REQUAL:
                        nc.scalar.dma_start(out=wqkb_sb, in_=wqkb_r)
                    for ci in range(NCI):
                        for hp in range(HSPLIT, HP):
                            nc.tensor.matmul(
                                ps0[ci],
                                lhsT=wqk_sb[:, hp, :, ci * 128:(ci + 1) * 128],
                                rhs=xt_sb[:, hp, :, 0:QSL],
                                start=False,
                                stop=(hp == HP - 1),
                                perf_mode=DR,
                            )
                        if ci % 2 == 0:
                            nc.scalar.activation(
                                out=qkt_sb[:, ci, 0:QSL],
                                in_=ps0[ci],
                                func=AFT.Identity,
                                bias=bqk_sb[:, ci:ci + 1],
                                scale=1.0,
                            )
                        else:
                            nc.vector.tensor_scalar_add(
                                out=qkt_sb[:, ci, 0:QSL],
                                in0=ps0[ci],
                                scalar1=bqk_sb[:, ci:ci + 1],
                            )

                # ---- phase 1a': Qt/Kt slices 1..3, slice-outer so each slice
                # only waits on its own xt DMA; ci-chains keep one PSUM bank
                # busy for 8 matmuls so bank drains overlap compute
                    for ss in range(1, N_QS):
                        for ci in range(NCI):
                            ps = ps_p1a.tile([128, QSL], f32, tag="psp1a", name=f"qk_{ci}_{ss}")
                            for hp in range(HP):
                                nc.tensor.matmul(
                                    ps,
                                    lhsT=wqk_sb[:, hp, :, ci * 128:(ci + 1) * 128],
                                    rhs=xt_sb[:, hp, :, ss * QSL:(ss + 1) * QSL],
                                    start=(hp == 0),
                                    stop=(hp == HP - 1),
                                    perf_mode=DR,
                                )
                            nc.scalar.activation(
                                out=qkt_sb[:, ci, ss * QSL:(ss + 1) * QSL],
                                in_=ps,
                                func=AFT.Identity,
                                bias=bqk_sb[:, ci:ci + 1],
                                scale=1.0,
                            )

                    # ---- phase 1b: V = Xt^T @ Wv (no bias; folded on host).
                    # s-tile 0 (keys 0-127) runs a separate bf16 chain: its
                    # values feed the early-query rows where fp8 error does
                    # not average out; both vb (bf16) and v_sb[0] (fp8) drain
                    # from it.
                    for sg in range(4):
                        jlist = list(range(1, 4)) if sg == 0 else list(range(4))
                        psv = {
                            j: ps_p1a.tile([128, HPC * 128], f32, tag="psp1a", name=f"v_{sg}_{j}")
                            for j in jlist
                        }
                        for hp in range(HP):
                            for j in jlist:
                                sti = sg * 4 + j
                                nc.tensor.matmul(
                                    psv[j],
                                    lhsT=xt_sb[:, hp, :, sti * 128:(sti + 1) * 128],
                                    rhs=wv_sb[:, hp, :, :],
                                    start=(hp == 0),
                                    stop=(hp == HP - 1),
                                    perf_mode=DR,
                                )
                        if sg == 0:
                            psvb = ps_p1a.tile([128, HPC * 128], f32, tag="psp1a", name="v_bf")
                            for ht in range(N_HT):
                                nc.tensor.matmul(
                                    psvb,
                                    lhsT=xtb_sb[:, ht, :],
                                    rhs=wvb_sb[:, ht, :],
                                    start=(ht == 0),
                                    stop=(ht == N_HT - 1),
                                )
                        # drain j=3 first: its bank is the one phase 2's
                        # first scores matmul aliases, so free it earliest
                        for j in (3, 2, 1, 0):
                            sti = sg * 4 + j
                            if sg == 0 and j == 0:
                                nc.scalar.copy(out=vb_sb, in_=psvb)
                                nc.vector.tensor_copy(out=v_sb[:, 0, :], in_=psvb)
                            elif j % 2 == 0:
                                nc.vector.tensor_copy(out=v_sb[:, sti, :], in_=psv[j])
                            else:
                                nc.scalar.copy(out=v_sb[:, sti, :], in_=psv[j])

                    # ---- phase 1b': requalify Qt/Kt columns 0:128 in bf16.
                    # Early queries (q<128) average too few softmax terms for
                    # the fp8 projection error to cancel; kt=0 keys feed every
                    # query slice, so overwrite qkt[:, ci, 0:128] in place.
                    # Runs last in phase 1 so the wqkb DMA has time to land.
                    for ci in range(NCI if QK_REQUAL else 0):
                        psq = ps_p1a.tile([128, QSL], f32, tag="psp1a", name=f"qkb_{ci}")
                        for ht in range(N_HT):
                            nc.tensor.matmul(
                                psq[:, 0:128],
                                lhsT=wqkb_sb[:, ht, ci * 128:(ci + 1) * 128],
                                rhs=xtb_sb[:, ht, :],
                                start=(ht == 0),
                                stop=(ht == N_HT - 1),
                            )
                        if ci % 2 == 0:
                            nc.scalar.activation(
                                out=qkt_sb[:, ci, 0:128],
                                in_=psq[:, 0:128],
                                func=AFT.Identity,
                                bias=bqk_sb[:, ci:ci + 1],
                                scale=1.0,
                            )
                        else:
                            nc.vector.tensor_scalar_add(
                                out=qkt_sb[:, ci, 0:128],
                                in0=psq[:, 0:128],
                                scalar1=bqk_sb[:, ci:ci + 1],
                            )

            # p1pool closed: xt/wqk/wv SBUF space freed; wd loads during phase 2
            with tc.tile_pool(name="p2pool", bufs=1) as p2pool, \
                 tc.tile_pool(name="ebpool", bufs=2) as ebpool:
                wd_sb = p2pool.tile([128, CPD, 2, H], f8, tag="wd_sb", name="wd_sb")
                wdb_sb = p2pool.tile([128, HPC, H], bf, tag="wdb_sb", name="wdb_sb")
                nc.sync.dma_start(out=wd_sb, in_=wd_r)
                nc.sync.dma_start(out=wdb_sb, in_=wdb_r)

                # ================= phase 2: attention =================
                with (
                    tc.tile_pool(name="ps_st", bufs=4, space="PSUM") as ps_st,
                    tc.tile_pool(name="ps_ctx", bufs=2, space="PSUM") as ps_ctx,
                    tc.tile_pool(name="ps_sums", bufs=2, space="PSUM") as ps_sums,
                ):
                    for h in range(HPC):
                        q_ci, k_ci = 2 * h, 2 * h + 1
                        for qs in range(N_QS - 1, -1, -1):
                            ctx_ps = ps_ctx.tile([128, QSL], f32, tag="ctxps", name=f"ctx_{h}_{qs}")
                            sums_ps = ps_sums.tile([128, QSL], f32, tag="sumsps", name=f"sums_{h}_{qs}")
                            def scores_block(kt, off, e_out, st_ps=None, st_lo=None):
                                # st_ps/st_lo let two small diag blocks share
                                # one PSUM tile in disjoint column ranges
                                if st_ps is None:
                                    st_ps = ps_st.tile([128, QSL], f32, tag="stps", name=f"st_{h}_{qs}_{kt}")
                                lo = off if st_lo is None else st_lo
                                hi = lo + (QSL - off)
                                nc.tensor.matmul(
                                    st_ps[:, lo:hi],
                                    lhsT=qkt_sb[:, k_ci, kt * KTL:(kt + 1) * KTL],
                                    rhs=qkt_sb[:, q_ci, qs * QSL + off:(qs + 1) * QSL],
                                    start=True,
                                    stop=True,
                                )
                                nc.scalar.activation(
                                    out=e_out,
                                    in_=st_ps[:, lo:hi],
                                    func=AFT.Exp,
                                    bias=alibi_sb[:, h * N_KT + kt: h * N_KT + kt + 1],
                                    scale=ESC,
                                )
                                return st_ps

                            # full blocks in pairs: fp8 DoubleRow for both the
                            # ctx and sums matmuls (256 contraction rows each).
                            # One-pair emission lag: pair kp's DR matmuls are
                            # emitted after pair kp+1's scores so the ACT exp
                            # pipeline stays ahead of the tensor queue.
                            def flush_pair(kp, e_p):
                                nc.tensor.matmul(
                                    ctx_ps,
                                    lhsT=v_sb[:, 2 * kp:2 * kp + 2, h * 128:(h + 1) * 128],
                                    rhs=e_p,
                                    start=(kp == 0),
                                    stop=False,
                                    perf_mode=DR,
                                )
                                nc.tensor.matmul(
                                    sums_ps,
                                    lhsT=ones_f8,
                                    rhs=e_p,
                                    start=(kp == 0),
                                    stop=False,
                                    perf_mode=DR,
                                )
                            pend = []
                            for kp in range(2 * qs):
                                e_p = epool.tile([128, 2, QSL], f8, tag="epair", name=f"ep_{h}_{qs}_{kp}")
                                for i in range(2):
                                    kt = 2 * kp + i
                                    scores_block(kt, 0, e_p[:, i, :])
                                pend.append((kp, e_p))
                                if len(pend) > 2:
                                    flush_pair(*pend.pop(0))
                            # diagonal blocks: causal q-suffix ctx singles,
                            # sums as zero-filled fp8 DoubleRow pairs; the
                            # pending full pair flushes under dp=0's scores
                            # all four diag scores emit first so their exps
                            # pipeline under the pend-flush + ctx matmuls
                            eb = None
                            e_d = []
                            st1 = None
                            for dp in range(2):
                                e_p = epool.tile([128, 2, QSL], f8, tag="epair", name=f"edp_{h}_{qs}_{dp}")
                                e_d.append(e_p)
                                for i in range(2):
                                    j = 2 * dp + i
                                    kt = 4 * qs + j
                                    off = j * 128
                                    if qs == 0 and j == 0:
                                        # early-query split: E[k<128, q<128] in
                                        # bf16 (eb); its slot in e_p zeroed so
                                        # the fp8 sums DoubleRow skips it
                                        eb = ebpool.tile([128, 128], bf, tag="eb", name=f"eb_{h}")
                                        st_ps = ps_st.tile([128, QSL], f32, tag="stps", name=f"st_{h}_0_0")
                                        nc.tensor.matmul(
                                            st_ps,
                                            lhsT=qkt_sb[:, k_ci, 0:KTL],
                                            rhs=qkt_sb[:, q_ci, 0:QSL],
                                            start=True,
                                            stop=True,
                                        )
                                        nc.scalar.activation(
                                            out=eb,
                                            in_=st_ps[:, 0:128],
                                            func=AFT.Exp,
                                            bias=alibi_sb[:, h * N_KT: h * N_KT + 1],
                                            scale=ESC,
                                        )
                                        nc.scalar.activation(
                                            out=e_p[:, 0, 128:QSL],
                                            in_=st_ps[:, 128:QSL],
                                            func=AFT.Exp,
                                            bias=alibi_sb[:, h * N_KT: h * N_KT + 1],
                                            scale=ESC,
                                        )
                                        nc.vector.memset(e_p[:, 0, 0:128], 0.0)
                                        nc.vector.tensor_mul(out=eb, in0=eb, in1=tri_sb)
                                        continue
                                    if off > 0:
                                        nc.vector.memset(e_p[:, i, 0:off], 0.0)
                                    if j == 3 and st1 is not None:
                                        # j=3 (128 cols) shares j=1's PSUM
                                        # tile in its unused low columns (j=1
                                        # retired two matmuls ago, no WAR)
                                        scores_block(kt, off, e_p[:, i, off:QSL],
                                                     st_ps=st1, st_lo=0)
                                    else:
                                        st = scores_block(kt, off, e_p[:, i, off:QSL])
                                        if j == 1:
                                            st1 = st
                                    nc.vector.tensor_mul(
                                        out=e_p[:, i, off:off + 128],
                                        in0=e_p[:, i, off:off + 128],
                                        in1=tri_sb,
                                    )
                            if pend:
                                flush_pair(*pend.pop(0))
                            for dp in range(2):
                                for i in range(2):
                                    j = 2 * dp + i
                                    kt = 4 * qs + j
                                    off = j * 128
                                    if dp == 1 and i == 0 and pend:
                                        # second pending pair flushes between
                                        # the two diag ctx sub-batches, before
                                        # j=3's stop closes the accumulation
                                        flush_pair(*pend.pop(0))
                                    if qs == 0 and j == 0:
                                        # bf16 ctx for q<128 (keys 0-127) plus
                                        # fp8 ctx for the rest of the block
                                        nc.tensor.matmul(
                                            ctx_ps[:, 0:128],
                                            lhsT=vb_sb[:, h * 128:(h + 1) * 128],
                                            rhs=eb,
                                            start=True,
                                            stop=False,
                                        )
                                        nc.tensor.matmul(
                                            ctx_ps[:, 128:QSL],
                                            lhsT=v_sb[:, 0, h * 128:(h + 1) * 128],
                                            rhs=e_d[0][:, 0, 128:QSL],
                                            start=True,
                                            stop=False,
                                        )
                                        continue
                                    nc.tensor.matmul(
                                        ctx_ps[:, off:QSL],
                                        lhsT=v_sb[:, kt, h * 128:(h + 1) * 128],
                                        rhs=e_d[dp][:, i, off:QSL],
                                        start=False,
                                        stop=(j == 3),
                                    )
                            for dp in range(2):
                                nc.tensor.matmul(
                                    sums_ps,
                                    lhsT=ones_f8,
                                    rhs=e_d[dp],
                                    start=(qs == 0 and dp == 0),
                                    stop=(dp == 1),
                                    perf_mode=DR,
                                )
                                if qs == 0 and dp == 0:
                                    # bf16 sums for q<128 accumulate onto the
                                    # zeros the DoubleRow left there
                                    nc.tensor.matmul(
                                        sums_ps[:, 0:128],
                                        lhsT=ones_bf,
                                        rhs=eb,
                                        start=False,
                                        stop=False,
                                    )
                            recipb = rpool.tile([128, QSL], f32, tag="recipb", name=f"recipb_{h}_{qs}")
                            nc.vector.reciprocal_approx_fast(out=recipb, in_=sums_ps)
                            if qs == 0:
                                nc.vector.tensor_mul(
                                    out=ctxb_sb[:, h, :],
                                    in0=ctx_ps[:, 0:128],
                                    in1=recipb[:, 0:128],
                                )
                                nc.vector.tensor_mul(
                                    out=ctx_sb[:, h, 128:QSL],
                                    in0=ctx_ps[:, 128:QSL],
                                    in1=recipb[:, 128:QSL],
                                )
                            else:
                                nc.vector.tensor_mul(
                                    out=ctx_sb[:, h, qs * QSL:(qs + 1) * QSL],
                                    in0=ctx_ps,
                                    in1=recipb,
                                )

                    # ============= phase 3: dense partial =============
                    # Lives inside the phase-2 pool scope and draws its PSUM
                    # tiles from ps_st, so there is no pool-transition barrier
                    # between the last softmax and the first dense matmul.
                    # Descending so: the first tiles depend on qs=3 ctx (ready
                    # early in h=3's loop); so=0 (bf16, needs the final ctxb
                    # write) lands last with its DVE producer long done.
                    dpools = [(ps_st, "stps"), (ps_ctx, "ctxps"), (ps_sums, "sumsps")]
                    for oi, so in enumerate(range(N_KT - 1, -1, -1)):
                        ot = outstage.tile([128, 4, QSL], bf, tag="ostage", name=f"o_{so}")
                        for hs in range(4):
                            # rotate psd over all three phase-2 pools: 8 banks
                            # of drain depth, like the old dedicated pool
                            dpool, dtag = dpools[(oi * 4 + hs) % 3]
                            psd = dpool.tile([128, QSL], f32, tag=dtag, name=f"d_{so}_{hs}")
                            if so == 0:
                                # early rows: bf16 ctx x bf16 wd (both only
                                # x32 once via v, so drain scale is 1/32)
                                for ct in range(HPC):
                                    nc.tensor.matmul(
                                        psd,
                                        lhsT=ctxb_sb[:, ct, :],
                                        rhs=wdb_sb[:, ct, hs * QSL:(hs + 1) * QSL],
                                        start=(ct == 0),
                                        stop=(ct == HPC - 1),
                                    )
                            else:
                                for cp in range(CPD):
                                    nc.tensor.matmul(
                                        psd,
                                        lhsT=ctx_sb[:, 2 * cp:2 * cp + 2, so * 128:(so + 1) * 128],
                                        rhs=wd_sb[:, cp, :, hs * QSL:(hs + 1) * QSL],
                                        start=(cp == 0),
                                        stop=(cp == CPD - 1),
                                        perf_mode=DR,
                                    )
                            dsc = (1.0 / WS) if so == 0 else DSC
                            # hs-outer: each drain starts right after its own
                            # matmul chain, overlapping the next chain
                            if hs % 2 == 0:
                                nc.vector.tensor_scalar_mul(out=ot[:, hs, :], in0=psd, scalar1=dsc)
                            else:
                                nc.scalar.mul(out=ot[:, hs, :], in_=psd, mul=dsc)
                            if so <= 2:
                                # final tiles: small per-hs DMAs rotating all
                                # three queues so the tail transfer drains fast
                                eng = (nc.sync, nc.scalar, nc.gpsimd)[(so * 4 + hs) % 3]
                                eng.dma_start(
                                    out=out_r[:, so, hs * QSL:(hs + 1) * QSL],
                                    in_=ot[:, hs, :],
                                )
                        if so > 2:
                            # rotate three DMA queues (sync/ACT HWDGE + gpsimd
                            # SWDGE): the fp8 DoubleRow compute outruns two
                            eng = (nc.sync, nc.scalar, nc.gpsimd)[oi % 3]
                            eng.dma_start(out=out_r[:, so, :], in_=ot)

    nc.compile()
    return nc


def _prepare_core_inputs(inputs):
    hs = np.asarray(inputs["hidden_states"], np.float32)
    alibi = np.asarray(inputs["alibi"], np.float32).reshape(B, NH, S)
    mask = np.asarray(inputs["attention_mask"], bool)
    W_qkv = np.asarray(inputs["W_qkv"], np.float32).reshape(H, NH, 3, HD)
    b_qkv = np.asarray(inputs["b_qkv"], np.float32).reshape(NH, 3, HD)
    W_dense = np.asarray(inputs["W_dense"], np.float32)

    # kernel is specialized for the causal mask (True = disallowed)
    causal = np.triu(np.ones((S, S), bool), k=1)
    for b in range(B):
        assert np.array_equal(mask[b, 0], causal), "non-causal mask"

    # 128x128 lower-triangle allow pattern (q >= k within a diagonal block)
    tri = np.tril(np.ones((128, 128), np.float32)).T.astype(bf16)
    tri = np.ascontiguousarray(tri)  # [k, q] layout: allow q' >= k'

    def pre_h(a):
        """[H, C] -> [128, HP*2*C]: SBUF layout, contiguous per-partition rows."""
        c = a.shape[1]
        return np.ascontiguousarray(
            a.reshape(HP, 2, 128, c).transpose(2, 0, 1, 3).reshape(128, -1))

    xtT = [np.ascontiguousarray(hs[b].T) for b in range(B)]
    # xt pre-arranged slice-major: [128, (ss hp two s')] so each 512-wide
    # query-slice DMA reads 8KB contiguous per partition
    xt = [np.ascontiguousarray(
        x.astype(f8e4).reshape(HP, 2, 128, N_QS, QSL)
        .transpose(2, 3, 0, 1, 4).reshape(128, -1)) for x in xtT]
    xtb = [np.ascontiguousarray(x[:, 0:128]).astype(bf16) for x in xtT]

    in_maps = []
    for c in range(NCORES):
        b, g = divmod(c, TP)
        heads = range(HPC * g, HPC * g + HPC)
        wqk = np.empty((H, NCI * 128), np.float32)
        bqk = np.empty((NCI, 128), np.float32)
        wv = np.empty((H, HPC * 128), np.float32)
        wd = np.empty((HPC * 128, H), np.float32)
        al = np.empty((128, HPC * N_KT), np.float32)
        for i, hh in enumerate(heads):
            wqk[:, (2 * i) * 128:(2 * i + 1) * 128] = W_qkv[:, hh, 0, :] * WS
            wqk[:, (2 * i + 1) * 128:(2 * i + 2) * 128] = W_qkv[:, hh, 1, :] * WS
            bqk[2 * i] = b_qkv[hh, 0, :] * WS
            bqk[2 * i + 1] = b_qkv[hh, 1, :] * WS
            wv[:, i * 128:(i + 1) * 128] = W_qkv[:, hh, 2, :] * WS
            wd[i * 128:(i + 1) * 128, :] = W_dense[hh * HD:(hh + 1) * HD, :] * WS
            al[:, i * N_KT:(i + 1) * N_KT] = (
                alibi[b, hh].reshape(N_KT, 128).T - np.log(16.0)
            )
        m = {
            "xt": xt[b],
            "wqk": pre_h(wqk.astype(f8e4)),
            "wv": pre_h(wv.astype(f8e4)),
            "wd": wd.astype(f8e4),
            "xtb": xtb[b],
            "wqkb": wqk.astype(bf16),
            "wvb": wv.astype(bf16),
            "wdb": (wd / WS).astype(bf16),
            "bqk": np.ascontiguousarray(bqk.T),
            "alibi": al,
            "tri": tri,
        }
        in_maps.append(m)
    return in_maps


def _run(inputs, trace=False, trace_cores=None):
    from concourse.bass_utils import run_bass_kernel_spmd

    in_maps = _prepare_core_inputs(inputs)
    if "prog" not in _program_cache:
        _program_cache["prog"] = _build_program()
    nc = _program_cache["prog"]
    res = run_bass_kernel_spmd(
        nc,
        in_maps,
        core_ids=list(range(NCORES)),
        trace=trace,
        trace_cores=trace_cores,
    )

    residual = np.asarray(inputs["residual"], np.float32)
    b_dense = np.asarray(inputs["b_dense"], np.float32)
    b_qkv = np.asarray(inputs["b_qkv"], np.float32).reshape(NH, 3, HD)
    W_dense = np.asarray(inputs["W_dense"], np.float32)
    # V bias passes through the softmax average: ctx = E@(xWv)/sums + bv,
    # so its dense image is a constant row added here.
    bv_full = b_qkv[:, 2, :].reshape(H)
    host_bias = b_dense + bv_full @ W_dense

    out = np.empty((B, S, H), np.float32)
    for b in range(B):
        acc = res.results[b * TP + 0]["out"].astype(np.float32)
        for g in range(1, TP):
            acc += res.results[b * TP + g]["out"].astype(np.float32)
        out[b] = acc + host_bias[None, :] + residual[b]
    return out, res


def kernel(**inputs) -> np.ndarray:
    out, _ = _run(inputs, trace=False)
    return out
